# revision 6
# baseline (speedup 1.0000x reference)
"""Trainium2 Bass kernel for DiceLoss (nn_DiceLoss_12326556140285).

Full (unsharded) contract: kernel(input, target, std) -> scalar np.ndarray.
Data-parallel over batch: 64 samples -> 8 cores x 8 samples. Inputs are
shipped to HBM as fp16 (verified: casting inputs moves the reference value
by ~1.7e-4 rel, far under the 2e-2 gate), zero-padded from 8192 to 8256
columns so every PE tile (127 data cols) is uniform.

Math (per sample, z = (x - thr)/std, thr = 0.9*max(target)):
  s = sigmoid(z) = (1 + w)/2,  w = tanh(z/2)
  t = target > thr ;  H = w > 0 ; r = relu(w) ; q = sign(target - thr) = 2t-1
  x = where(H == t, t, s)
  num = St + StH + Stw - Str + 1e-5
  den = 1.5 St + 0.5 SH + 0.5 Sr + 0.5 Stw - Str + 1e-5
  loss_b = 1 - num/den ;  output = mean_b loss_b
  with  St = (Sq+N)/2, Stw = (Sqw+Sw)/2, Str = (Sqr+Sr)/2, StH = (SqH+SH)/2.

Phantom columns: the last tile's 63 unused columns process zero-padded
data: q_ph = sign(0-thr) = -1, w_ph = tanh(-thr/(2 std)) = -T, r_ph = 0,
H_ph = 0. Their pollution of Sqw (+8064 T) cancels exactly against their
pollution of Sw (-8064 T) inside Stw = (Sqw+Sw)/2; Sq gains -8064 which is
folded into the St constant. Everything else is untouched.

Software pipelining: iteration b prefetches tgt[b+1] (DMA first in queue)
and computes thr[b+1] on DVE before sample b's r/H chunk passes, so the
ACT engine (the bottleneck: sign+tanh = ~15 us/sample) never waits for a
threshold. A dummy activation at the top triggers the ACT table load
during the first target DMA.

Engine split per core:
  ACT: w = tanh((x-thr)/(2 std)) and q = sign(tgt-thr), accum -> Sq.
  DVE: per-sample max via tensor-tensor max fold tree (2x mode), r = relu(w)
       and H = (w>0) as dense fp16 4x passes, PSUM extractions.
  PE : stationary tiles = ones column + 127 cols of q; moving = [w|r|H]
       (zero pad col + 127 data cols per tile). PSUM diag -> Sqw/Sqr/SqH,
       PSUM row 0 (ones row) -> Sw/Sr/SH column sums.
  GpSimd: 128-partition max for thr, final atom all-reduce.
"""

import numpy as np

N_CORES = 8
B = 64
SPC = B // N_CORES            # samples per core
FREE = 1024 * 1024 // 128     # 8192 real elements per partition per sample
TCOL = 127                    # data columns per PE tile (col 0 = ones/pad)
NT = 65                       # tiles per sample; 65*127 = 8255
PFREE = 8256                  # padded free size shipped to HBM
QW = NT * 128                 # 8320: q' buffer width
BW = 33 * 128                 # 4224: per-block width of a wrh chunk buffer
NEL = float(128 * FREE)       # real elements per sample
NPH = 63 * 128                # phantom elements per sample
N_ATOM = 7                    # Sqw,Sqr,SqH,Sw,Sr,SH,Sq
CHUNKS = [(0, 33), (33, 32)]            # (first tile, n tiles) samples 0..6
CHUNKS_LAST = [(0, 17), (17, 16), (33, 16), (49, 8), (57, 8)]   # sample 7

_COMPILED = {}


def build_nc(samples=SPC):
    import concourse.bass as bass
    import concourse.tile as tile
    from concourse import bacc, mybir, bass_isa

    f32 = mybir.dt.float32
    f16 = mybir.dt.float16
    Alu = mybir.AluOpType
    Act = mybir.ActivationFunctionType

    nc = bacc.Bacc("TRN2", target_bir_lowering=False, debug=False)
    inp_d = nc.dram_tensor("inp", [samples, 128, PFREE], f16, kind="ExternalInput").ap()
    tgt_d = nc.dram_tensor("tgt", [samples, 128, PFREE], f16, kind="ExternalInput").ap()
    std_d = nc.dram_tensor("std", [128, 1], f32, kind="ExternalInput").ap()
    eye_d = nc.dram_tensor("eye", [128, 128], f32, kind="ExternalInput").ap()
    out_d = nc.dram_tensor("out", [1, 1], f32, kind="ExternalOutput").ap()

    with tile.TileContext(nc) as tc:
        with (
            tc.tile_pool(name="const", bufs=1) as p_const,
            tc.tile_pool(name="psum", bufs=2, space="PSUM") as p_psum,
        ):
            # ---- persistent buffers (manual rotation) ----
            tgt_sb = [p_const.tile([128, PFREE], f16, name=f"tgt{i}") for i in range(2)]
            q_sb = [p_const.tile([128, QW], f16, name=f"q{i}") for i in range(2)]
            inp_sb = [p_const.tile([128, BW], f16, name=f"inp{i}") for i in range(3)]
            wrh = [p_const.tile([128, 3, BW], f16, name=f"wrh{i}") for i in range(3)]
            fold = p_const.tile([128, FREE], f16)
            eye = p_const.tile([128, 128], f32)
            junk = p_const.tile([128, 128], f32)
            atoms = p_const.tile([128, samples * N_ATOM], f32)
            dmy = p_const.tile([1, 1], f32)

            # first loads of sample 0 go out before the constants
            nc.sync.dma_start(tgt_sb[0][:, 0:2048], tgt_d[0][:, 0:2048])
            nc.sync.dma_start(tgt_sb[0][:, 2048:4096], tgt_d[0][:, 2048:4096])
            nc.sync.dma_start(tgt_sb[0][:, 4096:6144], tgt_d[0][:, 4096:6144])
            nc.sync.dma_start(tgt_sb[0][:, 6144:8256], tgt_d[0][:, 6144:8256])

            nc.sync.dma_start(eye[:], eye_d[:])
            nc.vector.memset(atoms[:], 0.0)
            nc.vector.memset(dmy[:], 0.0)
            # trigger the ACT function-table load right away
            nc.scalar.activation(dmy[:], dmy[:], Act.Tanh)

            # q': ones in col 0 of each tile
            for i in range(2):
                q4 = q_sb[i][:].rearrange("p (t l) -> p t l", l=128)
                nc.vector.memset(q4[:, :, 0:1], 1.0)
            # wrh: zero the w-block pad col of each tile (never written;
            # r/H passes rewrite their full blocks each use)
            for i in range(3):
                w4 = wrh[i][:, 0, :].rearrange("p (t l) -> p t l", l=128)
                nc.vector.memset(w4[:, :, 0:1], 0.0)

            # 1/(2 std), -0.9/(2 std) per partition (std replicated by host)
            std_sb = p_const.tile([128, 1], f32)
            nc.sync.dma_start(std_sb[:], std_d[:])
            std2 = p_const.tile([128, 1], f32)
            nc.vector.tensor_scalar_mul(std2[:], std_sb[:], 2.0)
            i2s = p_const.tile([128, 1], f32)
            nc.vector.reciprocal(i2s[:], std2[:])
            n09i2s = p_const.tile([128, 1], f32)
            nc.vector.tensor_scalar_mul(n09i2s[:], i2s[:], -0.9)

            nthr_t = [p_const.tile([128, 1], f32, name=f"nthr{i}") for i in range(2)]
            bias_t = [p_const.tile([128, 1], f32, name=f"bias{i}") for i in range(2)]
            mcol = [p_const.tile([128, 1], f32, name=f"mcol{i}") for i in range(2)]
            mall = [p_const.tile([128, 1], f32, name=f"mall{i}") for i in range(2)]

            def folds_and_thr(s, piecewise):
                """Max fold tree over tgt_sb[s%2] -> nthr/bias[s%2]."""
                A = s % 2
                tg = tgt_sb[A]
                if piecewise:
                    for p in range(4):
                        o = p * 2048
                        nc.vector.tensor_tensor(
                            fold[:, p * 1024 : (p + 1) * 1024],
                            tg[:, o : o + 1024], tg[:, o + 1024 : o + 2048],
                            Alu.max,
                        )
                    nc.vector.tensor_tensor(
                        fold[:, 4096:6144], fold[:, 0:2048], fold[:, 2048:4096],
                        Alu.max,
                    )
                    nc.vector.tensor_tensor(
                        fold[:, 6144:7168], fold[:, 4096:5120],
                        fold[:, 5120:6144], Alu.max,
                    )
                    nc.vector.tensor_tensor(
                        fold[:, 7168:7680], fold[:, 6144:6656],
                        fold[:, 6656:7168], Alu.max,
                    )
                    nc.vector.reduce_max(
                        out=mcol[A][:], in_=fold[:, 7168:7680],
                        axis=mybir.AxisListType.X,
                    )
                else:
                    nc.vector.tensor_tensor(
                        fold[:, 0:4096], tg[:, 0:4096], tg[:, 4096:8192], Alu.max
                    )
                    nc.vector.tensor_tensor(
                        fold[:, 4096:6144], fold[:, 0:2048], fold[:, 2048:4096],
                        Alu.max,
                    )
                    nc.vector.tensor_tensor(
                        fold[:, 6144:7168], fold[:, 4096:5120],
                        fold[:, 5120:6144], Alu.max,
                    )
                    nc.vector.tensor_tensor(
                        fold[:, 7168:7680], fold[:, 6144:6656],
                        fold[:, 6656:7168], Alu.max,
                    )
                    nc.vector.tensor_tensor(
                        fold[:, 7680:7936], fold[:, 7168:7424],
                        fold[:, 7424:7680], Alu.max,
                    )
                    nc.vector.reduce_max(
                        out=mcol[A][:], in_=fold[:, 7680:7936],
                        axis=mybir.AxisListType.X,
                    )
                nc.gpsimd.partition_all_reduce(
                    mall[A][:], mcol[A][:], channels=128,
                    reduce_op=bass_isa.ReduceOp.max,
                )
                # nthr = -0.9*max ; bias = -0.9*max/(2 std)
                nc.vector.tensor_scalar_mul(nthr_t[A][:], mall[A][:], -0.9)
                nc.vector.tensor_scalar(
                    bias_t[A][:], mall[A][:], n09i2s[:], None, Alu.mult
                )

            def extracts(ps, ab):
                """diag -> Sqw/Sqr/SqH ; ones-row 0 -> Sw/Sr/SH."""
                for blk in range(3):
                    nc.vector.scalar_tensor_tensor(
                        junk[:], ps[:, blk * 128 : (blk + 1) * 128], 1.0, eye[:],
                        Alu.mult, Alu.mult,
                        accum_out=atoms[:, ab + blk : ab + blk + 1],
                    )
                    nc.vector.reduce_sum(
                        out=atoms[0:1, ab + 3 + blk : ab + 4 + blk],
                        in_=ps[0:1, blk * 128 : (blk + 1) * 128],
                        axis=mybir.AxisListType.X,
                    )

            folds_and_thr(0, piecewise=True)

            g = 0          # global chunk counter (wrh/inp buffer rotation)
            ps_prev = None
            for b in range(samples):
                A = b % 2
                ab = b * N_ATOM
                tgt = tgt_sb[A]
                qb = q_sb[A]

                # prefetch next target; its DMA precedes this sample's input
                if b + 1 < samples:
                    nc.sync.dma_start(tgt_sb[(b + 1) % 2][:], tgt_d[b + 1])

                # ---- ACT: q = sign(tgt - thr), one call, 65 tiles (+Sq) ----
                qv = qb[:].rearrange("p (t l) -> p t l", l=128)
                tv = tgt[:, 0 : NT * TCOL].rearrange("p (t l) -> p t l", l=TCOL)
                nc.scalar.activation(
                    qv[:, :, 1:128], tv, Act.Sign, bias=nthr_t[A][:],
                    accum_out=atoms[:, ab + 6 : ab + 7],
                )

                # previous sample's PSUM extraction, then next threshold
                if ps_prev is not None:
                    extracts(ps_prev, ab - N_ATOM)
                if b + 1 < samples:
                    folds_and_thr(b + 1, piecewise=False)

                # ---- per chunk: load input, tanh -> w, relu/step, matmul ----
                ps = p_psum.tile([128, 384], f32)
                chunks = CHUNKS_LAST if b == samples - 1 else CHUNKS
                for (t0, ntc) in chunks:
                    W = g % 3
                    g += 1
                    wb = wrh[W]
                    ib = inp_sb[W]
                    ccols = ntc * TCOL
                    base = t0 * TCOL
                    nc.sync.dma_start(
                        ib[:, 0:ccols], inp_d[b][:, base : base + ccols]
                    )
                    # w = tanh((x - thr)/(2 std)) into 127-col tiles
                    wv = wb[:, 0, 0 : ntc * 128].rearrange(
                        "p (t l) -> p t l", l=128
                    )
                    iv = ib[:, 0:ccols].rearrange("p (t l) -> p t l", l=TCOL)
                    nc.scalar.activation(
                        wv[:, :, 1:128], iv, Act.Tanh,
                        bias=bias_t[A][:], scale=i2s[:],
                    )
                    # r = relu(w), H = (w > 0): dense full-block 4x passes
                    bw_c = ntc * 128
                    nc.vector.tensor_scalar(
                        wb[:, 1, 0:bw_c], wb[:, 0, 0:bw_c], 0.0, None, Alu.max
                    )
                    nc.vector.tensor_scalar(
                        wb[:, 2, 0:bw_c], wb[:, 0, 0:bw_c], 0.0, None, Alu.is_gt
                    )
                    # PE: psum[j1,j2] += sum_k q'[k,j1] * [w|r|H][k,j2]
                    for lt in range(ntc):
                        ti = t0 + lt
                        nc.tensor.matmul(
                            ps[:],
                            qb[:, ti * 128 : (ti + 1) * 128],
                            wb[:, :, lt * 128 : (lt + 1) * 128],
                            start=(ti == 0),
                            stop=(ti == NT - 1),
                        )
                ps_prev = ps

            extracts(ps_prev, (samples - 1) * N_ATOM)

            # ---- final reduction & loss assembly ----
            p_fin = p_const
            allat = p_fin.tile([128, samples * N_ATOM], f32)
            nc.gpsimd.partition_all_reduce(
                allat[:], atoms[:], channels=128,
                reduce_op=bass_isa.ReduceOp.add,
            )
            a = allat[0:1, :].rearrange("p (b k) -> p b k", k=N_ATOM)
            Sqw, Sqr, SqH, Sw, Sr, SH, Sq = (a[:, :, j] for j in range(N_ATOM))

            _tvn = [0]

            def tv2():
                _tvn[0] += 1
                return p_fin.tile([1, samples], f32, name=f"fin{_tvn[0]}")

            # St = (Sq + NEL + NPH)/2   (NPH corrects the phantom -1 signs)
            St = tv2(); nc.vector.tensor_scalar(
                St[:], Sq, 0.5, (NEL + NPH) / 2.0, Alu.mult, Alu.add
            )
            Stw2 = tv2(); nc.vector.tensor_add(Stw2[:], Sqw, Sw)   # 2*Stw
            Str2 = tv2(); nc.vector.tensor_add(Str2[:], Sqr, Sr)   # 2*Str
            StH2 = tv2(); nc.vector.tensor_add(StH2[:], SqH, SH)   # 2*StH

            # num = St + (StH2 + Stw2 - Str2)/2 + 1e-5
            n1 = tv2(); nc.vector.tensor_add(n1[:], StH2[:], Stw2[:])
            n2 = tv2(); nc.vector.tensor_sub(n2[:], n1[:], Str2[:])
            n3 = tv2(); nc.vector.tensor_scalar(
                n3[:], n2[:], 0.5, 1e-5, Alu.mult, Alu.add
            )
            num = tv2(); nc.vector.tensor_add(num[:], n3[:], St[:])

            # den = 1.5 St + 0.5 SH + 0.5 Sr + 0.25*Stw2 - 0.5*Str2 + 1e-5
            d1 = tv2(); nc.vector.tensor_scalar_mul(d1[:], St[:], 1.5)
            d2 = tv2(); nc.vector.tensor_add(d2[:], SH, Sr)
            d3 = tv2(); nc.vector.tensor_scalar(
                d3[:], d2[:], 0.5, 1e-5, Alu.mult, Alu.add
            )
            d4 = tv2(); nc.vector.tensor_scalar_mul(d4[:], Stw2[:], 0.25)
            d5 = tv2(); nc.vector.tensor_scalar_mul(d5[:], Str2[:], 0.5)
            d6 = tv2(); nc.vector.tensor_add(d6[:], d1[:], d3[:])
            d7 = tv2(); nc.vector.tensor_add(d7[:], d6[:], d4[:])
            den = tv2(); nc.vector.tensor_sub(den[:], d7[:], d5[:])

            rv = tv2(); nc.vector.reciprocal(rv[:], den[:])
            pv = tv2(); nc.vector.tensor_mul(pv[:], num[:], rv[:])
            sv = p_fin.tile([1, 1], f32, name="finsum")
            nc.vector.reduce_sum(out=sv[:], in_=pv[:], axis=mybir.AxisListType.X)
            # sum_b (1 - pv_b) / B  (partial over this core's samples)
            outsb = p_fin.tile([1, 1], f32, name="finout")
            nc.vector.tensor_scalar(
                outsb[:], sv[:], -1.0 / B, float(samples) / B, Alu.mult, Alu.add
            )
            nc.sync.dma_start(out_d[:], outsb[:])

    nc.compile()
    return nc


def _get_compiled():
    if "nc" not in _COMPILED:
        _COMPILED["nc"] = build_nc()
    return _COMPILED["nc"]


def make_in_maps(input, target, std):
    inp = np.asarray(input).reshape(B, 128, FREE).astype(np.float16)
    tgt = np.asarray(target).reshape(B, 128, FREE).astype(np.float16)
    pad = ((0, 0), (0, 0), (0, PFREE - FREE))
    inp = np.pad(inp, pad)
    tgt = np.pad(tgt, pad)
    stdv = np.full((128, 1), np.asarray(std, dtype=np.float32).reshape(-1)[0],
                   dtype=np.float32)
    eye = np.eye(128, dtype=np.float32)
    in_maps = []
    for c in range(N_CORES):
        sl = slice(c * SPC, (c + 1) * SPC)
        in_maps.append({
            "inp": np.ascontiguousarray(inp[sl]),
            "tgt": np.ascontiguousarray(tgt[sl]),
            "std": stdv,
            "eye": eye,
        })
    return in_maps


def kernel(input, target, std):
    from concourse.bass_utils import run_bass_kernel_spmd

    nc = _get_compiled()
    in_maps = make_in_maps(input, target, std)
    res = run_bass_kernel_spmd(nc, in_maps, list(range(N_CORES)))
    total = np.float32(0.0)
    for c in range(N_CORES):
        total += np.float32(res.results[c]["out"][0, 0])
    return np.array(total, dtype=np.float32)


# revision 7
# speedup vs baseline: 1.2053x; 1.2053x over previous
"""Trainium2 Bass kernel for DiceLoss (nn_DiceLoss_12326556140285).

Full (unsharded) contract: kernel(input, target, std) -> scalar np.ndarray.
Data-parallel over batch: 64 samples -> 8 cores x 8 samples. Inputs are
shipped to HBM as fp16 (verified: casting inputs moves the reference value
by ~1.7e-4 rel, far under the 2e-2 gate), zero-padded from 8192 to 8256
columns so every PE tile (127 data cols) is uniform.

Math (per sample, z = (x - thr)/std, thr = 0.9*max(target)):
  s = sigmoid(z) = (1 + w)/2,  w = tanh(z/2)
  t = target > thr ;  H = w > 0 ; r = relu(w) ; q = sign(target - thr) = 2t-1
  x = where(H == t, t, s)
  num = St + StH + Stw - Str + 1e-5
  den = 1.5 St + 0.5 SH + 0.5 Sr + 0.5 Stw - Str + 1e-5
  loss_b = 1 - num/den ;  output = mean_b loss_b
  with  St = (Sq+N)/2, Stw = (Sqw+Sw)/2, Str = (Sqr+Sr)/2, StH = (SqH+SH)/2.

Phantom columns: the last tile's 63 unused columns process zero-padded
data: q_ph = sign(0-thr) = -1, w_ph = tanh(-thr/(2 std)) = -T, r_ph = 0,
H_ph = 0. Their pollution of Sqw (+8064 T) cancels exactly against their
pollution of Sw (-8064 T) inside Stw = (Sqw+Sw)/2; Sq gains -8064 which is
folded into the St constant. Everything else is untouched.

Software pipelining: iteration b prefetches tgt[b+1] (DMA first in queue)
and computes thr[b+1] on DVE before sample b's r/H chunk passes, so the
ACT engine (the bottleneck: sign+tanh = ~15 us/sample) never waits for a
threshold. A dummy activation at the top triggers the ACT table load
during the first target DMA.

Engine split per core:
  ACT: w = tanh((x-thr)/(2 std)) and q = sign(tgt-thr), accum -> Sq.
  DVE: per-sample max via tensor-tensor max fold tree (2x mode), r = relu(w)
       and H = (w>0) as dense fp16 4x passes, PSUM extractions.
  PE : stationary tiles = ones column + 127 cols of q; moving = [w|r|H]
       (zero pad col + 127 data cols per tile). PSUM diag -> Sqw/Sqr/SqH,
       PSUM row 0 (ones row) -> Sw/Sr/SH column sums.
  GpSimd: 128-partition max for thr, final atom all-reduce.
"""

import numpy as np

N_CORES = 8
B = 64
SPC = B // N_CORES            # samples per core
FREE = 1024 * 1024 // 128     # 8192 real elements per partition per sample
TCOL = 127                    # data columns per PE tile (col 0 = ones/pad)
NT = 65                       # tiles per sample; 65*127 = 8255
PFREE = 8256                  # padded free size shipped to HBM
QW = NT * 128                 # 8320: q' buffer width
BW = 33 * 128                 # 4224: per-block width of a wrh chunk buffer
NEL = float(128 * FREE)       # real elements per sample
NPH = 63 * 128                # phantom elements per sample
N_ATOM = 7                    # Sqw,Sqr,SqH,Sw,Sr,SH,Sq
CHUNKS = [(0, 33), (33, 32)]            # (first tile, n tiles) samples 0..6
CHUNKS_LAST = [(0, 17), (17, 16), (33, 16), (49, 16)]   # sample 7

_COMPILED = {}


def build_nc(samples=SPC):
    import concourse.bass as bass
    import concourse.tile as tile
    from concourse import bacc, mybir, bass_isa

    f32 = mybir.dt.float32
    f16 = mybir.dt.float16
    Alu = mybir.AluOpType
    Act = mybir.ActivationFunctionType

    nc = bacc.Bacc("TRN2", target_bir_lowering=False, debug=False)
    inp_d = nc.dram_tensor("inp", [samples, 128, PFREE], f16, kind="ExternalInput").ap()
    tgt_d = nc.dram_tensor("tgt", [samples, 128, PFREE], f16, kind="ExternalInput").ap()
    std_d = nc.dram_tensor("std", [128, 1], f32, kind="ExternalInput").ap()
    eye_d = nc.dram_tensor("eye", [128, 128], f32, kind="ExternalInput").ap()
    out_d = nc.dram_tensor("out", [1, 1], f32, kind="ExternalOutput").ap()

    with tile.TileContext(nc) as tc:
        with (
            tc.tile_pool(name="const", bufs=1) as p_const,
            tc.tile_pool(name="psum", bufs=2, space="PSUM") as p_psum,
        ):
            # ---- persistent buffers (manual rotation) ----
            tgt_sb = [p_const.tile([128, PFREE], f16, name=f"tgt{i}") for i in range(2)]
            q_sb = [p_const.tile([128, QW], f16, name=f"q{i}") for i in range(2)]
            inp_sb = [p_const.tile([128, BW], f16, name=f"inp{i}") for i in range(3)]
            wrh = [p_const.tile([128, 3, BW], f16, name=f"wrh{i}") for i in range(3)]
            fold = p_const.tile([128, FREE], f16)
            eye = p_const.tile([128, 128], f32)
            junk = p_const.tile([128, 128], f32)
            atoms = p_const.tile([128, samples * N_ATOM], f32)
            dmy = p_const.tile([1, 1], f32)

            # first loads of sample 0 go out before the constants
            nc.sync.dma_start(tgt_sb[0][:, 0:2048], tgt_d[0][:, 0:2048])
            nc.sync.dma_start(tgt_sb[0][:, 2048:4096], tgt_d[0][:, 2048:4096])
            nc.sync.dma_start(tgt_sb[0][:, 4096:6144], tgt_d[0][:, 4096:6144])
            nc.sync.dma_start(tgt_sb[0][:, 6144:8256], tgt_d[0][:, 6144:8256])
            nc.sync.dma_start(tgt_sb[1][:], tgt_d[1])

            nc.sync.dma_start(eye[:], eye_d[:])
            nc.vector.memset(atoms[:], 0.0)
            nc.vector.memset(dmy[:], 0.0)
            # trigger the ACT function-table load right away
            nc.scalar.activation(dmy[:], dmy[:], Act.Tanh)

            # q': ones in col 0 of each tile
            for i in range(2):
                q4 = q_sb[i][:].rearrange("p (t l) -> p t l", l=128)
                nc.vector.memset(q4[:, :, 0:1], 1.0)
            # wrh: zero the w-block pad col of each tile (never written;
            # r/H passes rewrite their full blocks each use)
            for i in range(3):
                w4 = wrh[i][:, 0, :].rearrange("p (t l) -> p t l", l=128)
                nc.vector.memset(w4[:, :, 0:1], 0.0)

            # 1/(2 std), -0.9/(2 std) per partition (std replicated by host)
            std_sb = p_const.tile([128, 1], f32)
            nc.sync.dma_start(std_sb[:], std_d[:])
            std2 = p_const.tile([128, 1], f32)
            nc.vector.tensor_scalar_mul(std2[:], std_sb[:], 2.0)
            i2s = p_const.tile([128, 1], f32)
            nc.vector.reciprocal(i2s[:], std2[:])
            n09i2s = p_const.tile([128, 1], f32)
            nc.vector.tensor_scalar_mul(n09i2s[:], i2s[:], -0.9)

            nthr_t = [p_const.tile([128, 1], f32, name=f"nthr{i}") for i in range(2)]
            bias_t = [p_const.tile([128, 1], f32, name=f"bias{i}") for i in range(2)]
            mcol = [p_const.tile([128, 1], f32, name=f"mcol{i}") for i in range(2)]
            mall = [p_const.tile([128, 1], f32, name=f"mall{i}") for i in range(2)]

            def folds_and_thr(s, piecewise):
                """Max fold tree over tgt_sb[s%2] -> nthr/bias[s%2]."""
                A = s % 2
                tg = tgt_sb[A]
                if piecewise:
                    for p in range(4):
                        o = p * 2048
                        nc.vector.tensor_tensor(
                            fold[:, p * 1024 : (p + 1) * 1024],
                            tg[:, o : o + 1024], tg[:, o + 1024 : o + 2048],
                            Alu.max,
                        )
                    nc.vector.tensor_tensor(
                        fold[:, 4096:6144], fold[:, 0:2048], fold[:, 2048:4096],
                        Alu.max,
                    )
                    nc.vector.tensor_tensor(
                        fold[:, 6144:7168], fold[:, 4096:5120],
                        fold[:, 5120:6144], Alu.max,
                    )
                    nc.vector.tensor_tensor(
                        fold[:, 7168:7680], fold[:, 6144:6656],
                        fold[:, 6656:7168], Alu.max,
                    )
                    nc.vector.reduce_max(
                        out=mcol[A][:], in_=fold[:, 7168:7680],
                        axis=mybir.AxisListType.X,
                    )
                else:
                    nc.vector.tensor_tensor(
                        fold[:, 0:4096], tg[:, 0:4096], tg[:, 4096:8192], Alu.max
                    )
                    nc.vector.tensor_tensor(
                        fold[:, 4096:6144], fold[:, 0:2048], fold[:, 2048:4096],
                        Alu.max,
                    )
                    nc.vector.tensor_tensor(
                        fold[:, 6144:7168], fold[:, 4096:5120],
                        fold[:, 5120:6144], Alu.max,
                    )
                    nc.vector.tensor_tensor(
                        fold[:, 7168:7680], fold[:, 6144:6656],
                        fold[:, 6656:7168], Alu.max,
                    )
                    nc.vector.tensor_tensor(
                        fold[:, 7680:7936], fold[:, 7168:7424],
                        fold[:, 7424:7680], Alu.max,
                    )
                    nc.vector.reduce_max(
                        out=mcol[A][:], in_=fold[:, 7680:7936],
                        axis=mybir.AxisListType.X,
                    )
                nc.gpsimd.partition_all_reduce(
                    mall[A][:], mcol[A][:], channels=128,
                    reduce_op=bass_isa.ReduceOp.max,
                )
                # nthr = -0.9*max ; bias = -0.9*max/(2 std)
                nc.vector.tensor_scalar_mul(nthr_t[A][:], mall[A][:], -0.9)
                nc.vector.tensor_scalar(
                    bias_t[A][:], mall[A][:], n09i2s[:], None, Alu.mult
                )

            def extracts(ps, ab):
                """diag -> Sqw/Sqr/SqH ; ones-row 0 -> Sw/Sr/SH."""
                for blk in range(3):
                    nc.vector.scalar_tensor_tensor(
                        junk[:], ps[:, blk * 128 : (blk + 1) * 128], 1.0, eye[:],
                        Alu.mult, Alu.mult,
                        accum_out=atoms[:, ab + blk : ab + blk + 1],
                    )
                    nc.vector.reduce_sum(
                        out=atoms[0:1, ab + 3 + blk : ab + 4 + blk],
                        in_=ps[0:1, blk * 128 : (blk + 1) * 128],
                        axis=mybir.AxisListType.X,
                    )

            folds_and_thr(0, piecewise=True)

            g = 0          # global chunk counter (wrh/inp buffer rotation)
            ps_prev = None
            for b in range(samples):
                A = b % 2
                ab = b * N_ATOM
                tgt = tgt_sb[A]
                qb = q_sb[A]

                # ---- ACT: q = sign(tgt - thr), one call, 65 tiles (+Sq) ----
                qv = qb[:].rearrange("p (t l) -> p t l", l=128)
                tv = tgt[:, 0 : NT * TCOL].rearrange("p (t l) -> p t l", l=TCOL)
                nc.scalar.activation(
                    qv[:, :, 1:128], tv, Act.Sign, bias=nthr_t[A][:],
                    accum_out=atoms[:, ab + 6 : ab + 7],
                )

                # next sample's threshold (its target is already resident)
                if b + 1 < samples:
                    folds_and_thr(b + 1, piecewise=False)

                # ---- per chunk: load input, tanh -> w, relu/step, matmul ----
                ps = p_psum.tile([128, 384], f32)
                chunks = CHUNKS_LAST if b == samples - 1 else CHUNKS
                for (t0, ntc) in chunks:
                    W = g % 3
                    g += 1
                    wb = wrh[W]
                    ib = inp_sb[W]
                    ccols = ntc * TCOL
                    base = t0 * TCOL
                    nc.sync.dma_start(
                        ib[:, 0:ccols], inp_d[b][:, base : base + ccols]
                    )
                    # w = tanh((x - thr)/(2 std)) into 127-col tiles
                    wv = wb[:, 0, 0 : ntc * 128].rearrange(
                        "p (t l) -> p t l", l=128
                    )
                    iv = ib[:, 0:ccols].rearrange("p (t l) -> p t l", l=TCOL)
                    nc.scalar.activation(
                        wv[:, :, 1:128], iv, Act.Tanh,
                        bias=bias_t[A][:], scale=i2s[:],
                    )
                    # r = relu(w), H = (w > 0): dense full-block 4x passes
                    bw_c = ntc * 128
                    nc.vector.tensor_scalar(
                        wb[:, 1, 0:bw_c], wb[:, 0, 0:bw_c], 0.0, None, Alu.max
                    )
                    nc.vector.tensor_scalar(
                        wb[:, 2, 0:bw_c], wb[:, 0, 0:bw_c], 0.0, None, Alu.is_gt
                    )
                    # PE: psum[j1,j2] += sum_k q'[k,j1] * [w|r|H][k,j2]
                    for lt in range(ntc):
                        ti = t0 + lt
                        nc.tensor.matmul(
                            ps[:],
                            qb[:, ti * 128 : (ti + 1) * 128],
                            wb[:, :, lt * 128 : (lt + 1) * 128],
                            start=(ti == 0),
                            stop=(ti == NT - 1),
                        )
                # prefetch the target two samples ahead (after this
                # sample's input loads in the queue)
                if b + 2 < samples:
                    nc.sync.dma_start(tgt_sb[b % 2][:], tgt_d[b + 2])
                extracts(ps, ab)
                ps_prev = ps

            # ---- final reduction & loss assembly ----
            p_fin = p_const
            allat = p_fin.tile([128, samples * N_ATOM], f32)
            nc.gpsimd.partition_all_reduce(
                allat[:], atoms[:], channels=128,
                reduce_op=bass_isa.ReduceOp.add,
            )
            a = allat[0:1, :].rearrange("p (b k) -> p b k", k=N_ATOM)
            Sqw, Sqr, SqH, Sw, Sr, SH, Sq = (a[:, :, j] for j in range(N_ATOM))

            _tvn = [0]

            def tv2():
                _tvn[0] += 1
                return p_fin.tile([1, samples], f32, name=f"fin{_tvn[0]}")

            # St = (Sq + NEL + NPH)/2   (NPH corrects the phantom -1 signs)
            St = tv2(); nc.vector.tensor_scalar(
                St[:], Sq, 0.5, (NEL + NPH) / 2.0, Alu.mult, Alu.add
            )
            Stw2 = tv2(); nc.vector.tensor_add(Stw2[:], Sqw, Sw)   # 2*Stw
            Str2 = tv2(); nc.vector.tensor_add(Str2[:], Sqr, Sr)   # 2*Str
            StH2 = tv2(); nc.vector.tensor_add(StH2[:], SqH, SH)   # 2*StH

            # num = St + (StH2 + Stw2 - Str2)/2 + 1e-5
            n1 = tv2(); nc.vector.tensor_add(n1[:], StH2[:], Stw2[:])
            n2 = tv2(); nc.vector.tensor_sub(n2[:], n1[:], Str2[:])
            n3 = tv2(); nc.vector.tensor_scalar(
                n3[:], n2[:], 0.5, 1e-5, Alu.mult, Alu.add
            )
            num = tv2(); nc.vector.tensor_add(num[:], n3[:], St[:])

            # den = 1.5 St + 0.5 SH + 0.5 Sr + 0.25*Stw2 - 0.5*Str2 + 1e-5
            d1 = tv2(); nc.vector.tensor_scalar_mul(d1[:], St[:], 1.5)
            d2 = tv2(); nc.vector.tensor_add(d2[:], SH, Sr)
            d3 = tv2(); nc.vector.tensor_scalar(
                d3[:], d2[:], 0.5, 1e-5, Alu.mult, Alu.add
            )
            d4 = tv2(); nc.vector.tensor_scalar_mul(d4[:], Stw2[:], 0.25)
            d5 = tv2(); nc.vector.tensor_scalar_mul(d5[:], Str2[:], 0.5)
            d6 = tv2(); nc.vector.tensor_add(d6[:], d1[:], d3[:])
            d7 = tv2(); nc.vector.tensor_add(d7[:], d6[:], d4[:])
            den = tv2(); nc.vector.tensor_sub(den[:], d7[:], d5[:])

            rv = tv2(); nc.vector.reciprocal(rv[:], den[:])
            pv = tv2(); nc.vector.tensor_mul(pv[:], num[:], rv[:])
            sv = p_fin.tile([1, 1], f32, name="finsum")
            nc.vector.reduce_sum(out=sv[:], in_=pv[:], axis=mybir.AxisListType.X)
            # sum_b (1 - pv_b) / B  (partial over this core's samples)
            outsb = p_fin.tile([1, 1], f32, name="finout")
            nc.vector.tensor_scalar(
                outsb[:], sv[:], -1.0 / B, float(samples) / B, Alu.mult, Alu.add
            )
            nc.sync.dma_start(out_d[:], outsb[:])

    nc.compile()
    return nc


def _get_compiled():
    if "nc" not in _COMPILED:
        _COMPILED["nc"] = build_nc()
    return _COMPILED["nc"]


def make_in_maps(input, target, std):
    inp = np.asarray(input).reshape(B, 128, FREE).astype(np.float16)
    tgt = np.asarray(target).reshape(B, 128, FREE).astype(np.float16)
    pad = ((0, 0), (0, 0), (0, PFREE - FREE))
    inp = np.pad(inp, pad)
    tgt = np.pad(tgt, pad)
    stdv = np.full((128, 1), np.asarray(std, dtype=np.float32).reshape(-1)[0],
                   dtype=np.float32)
    eye = np.eye(128, dtype=np.float32)
    in_maps = []
    for c in range(N_CORES):
        sl = slice(c * SPC, (c + 1) * SPC)
        in_maps.append({
            "inp": np.ascontiguousarray(inp[sl]),
            "tgt": np.ascontiguousarray(tgt[sl]),
            "std": stdv,
            "eye": eye,
        })
    return in_maps


def kernel(input, target, std):
    from concourse.bass_utils import run_bass_kernel_spmd

    nc = _get_compiled()
    in_maps = make_in_maps(input, target, std)
    res = run_bass_kernel_spmd(nc, in_maps, list(range(N_CORES)))
    total = np.float32(0.0)
    for c in range(N_CORES):
        total += np.float32(res.results[c]["out"][0, 0])
    return np.array(total, dtype=np.float32)
